# revision 1
# baseline (speedup 1.0000x reference)
"""APA (attribute propagation) on 8 trn2 NeuronCores.

out_{t+1} = spmm(D^-1/2 A D^-1/2, out_t); out_{t+1}[known] = x[known]; 10 iters.

y-space trick: with a = deg^-1/2 and y = a*out, the iteration is
  y_{t+1}[r] = a_r^2 * sum_{e: row_e=r, row!=col} y_t[col_e]
for unknown r; known rows of y are constant (a_k * x_k), so edges into known
destinations are dropped and no per-edge weight is needed.

Device design (dest-sharded across 8 cores, full y-table replicated):
- table [T,64] f32 internal DRAM per core; rows [0,8A) active dests
  (core-major), then constant rows. int16 gather indices reach only 32768
  rows, so the table is split into 4 windows; edges are host-sorted by
  (edge-rank t, source window w, dest).
- per iteration: dma_gather (window-pure calls) pulls source rows into an
  SBUF ring; dma_scatter_add (round-pure calls) accumulates them into a
  per-core DRAM accumulator. Within a round every destination appears at
  most once (t-th edge of each dest) so concurrent scatter descriptors
  never RMW-collide; rounds are serialized by semaphore.
- accumulator is read back, scaled by a^2 (DVE), written to a bounce
  buffer and AllGathered into every core's table active region.
All 8 cores run one identical instruction stream (SPMD); per-core data
(indices, scales) comes via input tensors, padded to uniform shapes.
"""

import numpy as np

N_CORES = 8
D = 64
P = 128
CALLMAX = 1920  # idxs per DMA call: 121 ring entries; queues alternate
N_ITERS = 10


# ---------------------------------------------------------------- host prep


def _prepare(x, edge_index, known_feature_mask):
    N = x.shape[0]
    row = edge_index[0].astype(np.int64)
    col = edge_index[1].astype(np.int64)

    deg = np.bincount(row, minlength=N)
    a = np.zeros(N, np.float32)
    nz = deg > 0
    a[nz] = (1.0 / np.sqrt(deg[nz].astype(np.float32))).astype(np.float32)

    is_known = np.zeros(N, bool)
    is_known[known_feature_mask] = True
    known_nodes = np.nonzero(is_known)[0]

    keep = (row != col) & (~is_known[row])
    krow = row[keep]
    kcol = col[keep]
    kd = np.bincount(krow, minlength=N)

    active_mask = (~is_known) & (kd > 0)
    act_nodes = np.nonzero(active_mask)[0]
    order = np.argsort(-kd[act_nodes], kind="stable")
    act_sorted = act_nodes[order]
    n_act = len(act_sorted)

    A = -(-n_act // N_CORES)
    A = -(-A // P) * P  # pad to multiple of 128 for clean tiles

    # dest_of[c][j] = node, ASCENDING degree within each core (-1 pad at
    # the low end): the low-degree half [0, A/2) finalizes after round
    # ~median-degree, letting its readback/AllGather overlap later rounds.
    dest = np.full((N_CORES, A), -1, np.int64)
    for c in range(N_CORES):
        lst = act_sorted[c::N_CORES][::-1]  # ascending degree
        dest[c, A - len(lst) :] = lst

    # table slots: half-split layout so each half is contiguous across
    # cores (AllGather piece k concatenates core shards of half k):
    # rows [0, 8H) = all cores' half-1 slots, [8H, 8A) = half-2.
    HT = A // 2
    inact_nodes = np.nonzero(~active_mask)[0]
    slot_of = np.full(N, -1, np.int64)
    for c in range(N_CORES):
        m = dest[c] >= 0
        jj = np.nonzero(m)[0]
        tslot = np.where(
            jj < HT, c * HT + jj, N_CORES * HT + c * HT + (jj - HT)
        )
        slot_of[dest[c, jj]] = tslot
    slot_of[inact_nodes] = N_CORES * A + np.arange(len(inact_nodes))
    T_rows = N_CORES * A + len(inact_nodes)
    WR = -(-T_rows // 4)  # window rows
    assert WR <= 32767, WR

    # per-core CSR by local dest slot
    eorder = np.argsort(krow, kind="stable")
    src_slot_sorted = slot_of[kcol[eorder]]  # grouped by dest node
    starts = np.zeros(N + 1, np.int64)
    starts[1:] = np.cumsum(kd)

    kd_dest = np.where(dest >= 0, kd[np.maximum(dest, 0)], 0)  # [C, A]
    max_deg = int(kd_dest.max())
    T1 = int(kd_dest[:, :HT].max())  # half-1 dests final after round T1-1

    # build per (region r, round t, window w) edge lists per core.
    # region 0: sources in the constant table rows (slot >= 8A) -- these
    # gathers don't depend on the AllGather and run during it.
    ACT_END = N_CORES * A
    cells = {}  # (r, t, w) -> list per core of (gidx_local, sidx_local)
    for t in range(max_deg):
        live = kd_dest > t  # [C, A]
        for c in range(N_CORES):
            js = np.nonzero(live[c])[0]
            if len(js) == 0:
                continue
            srcs = src_slot_sorted[starts[dest[c, js]] + t]
            regs = (srcs < ACT_END).astype(np.int64)
            ws = srcs // WR
            for r in range(2):
                for w in range(4):
                    m = (ws == w) & (regs == r)
                    cells.setdefault((r, t, w), [[] for _ in range(N_CORES)])
                    if m.any():
                        cells[(r, t, w)][c] = [srcs[m] - w * WR, js[m]]

    # uniform call schedule: for each (t, w) in order, n = max over cores,
    # rounded to 128; split into <= CALLMAX chunks.
    # schedule entries: (t, w, n_call)
    sched = []
    for r in range(2):
        for t in range(max_deg):
            for w in range(4):
                if (r, t, w) not in cells:
                    continue
                per_core = cells[(r, t, w)]
                n = max((len(e[0]) if e else 0) for e in per_core)
                if n == 0:
                    continue
                n = -(-n // 128) * 128
                o = 0
                while o < n:
                    nc_ = min(CALLMAX, n - o)
                    sched.append((r, t, w, nc_))
                    o += nc_

    NCALL = len(sched)
    SWI = sum(s[3] for s in sched)

    # per-core packed idx arrays (wrapped in 16 partitions, replicated x8)
    gidx16 = np.zeros((N_CORES, 16, SWI // 16), np.int16)
    sidx16 = np.zeros((N_CORES, 16, SWI // 16), np.int16)
    # round-parity double-buffered accumulator: consecutive rounds (in
    # schedule order) scatter into different halves of accum, so only
    # rounds two apart need a barrier.  Half stride A+P; trash row at A
    # within each half.
    HALF = A + P
    rounds_seq = []
    for (r, t, w, n) in sched:
        if (r, t) not in rounds_seq:
            rounds_seq.append((r, t))
    round_par = {rt: i % 2 for i, rt in enumerate(rounds_seq)}
    TRASH = A  # per-half trash row (offset added below)

    # fill: walk sched; keep per-(t,w) cursor into that cell's edges
    cursors = {}
    off = 0
    call_meta = []  # (r, t, w, n, off)
    for (r, t, w, n) in sched:
        cur = cursors.get((r, t, w), 0)
        for c in range(N_CORES):
            e = cells[(r, t, w)][c]
            if e:
                g_all, s_all = e[0], e[1]
            else:
                g_all = np.zeros(0, np.int64)
                s_all = np.zeros(0, np.int64)
            g = g_all[cur : cur + n]
            s = s_all[cur : cur + n]
            pad = n - len(g)
            if pad:
                g = np.concatenate([g, np.zeros(pad, np.int64)])
                s = np.concatenate([s, np.full(pad, TRASH, np.int64)])
            s = s + round_par[(r, t)] * HALF
            i = np.arange(n)
            gidx16[c, i % 16, (off + i) // 16] = g.astype(np.int16)
            sidx16[c, i % 16, (off + i) // 16] = s.astype(np.int16)
        call_meta.append((r, t, w, n, off))
        cursors[(r, t, w)] = cur + n
        off += n
    assert off == SWI

    gidx = np.tile(gidx16, (1, 8, 1))  # [C, 128, SWI//16]
    sidx = np.tile(sidx16, (1, 8, 1))

    # b scale, flat local-slot order, expanded over D
    bvals = np.where(dest >= 0, a[np.maximum(dest, 0)] ** 2, 0.0).astype(np.float32)
    bexp = np.repeat(bvals[:, :, None], D, axis=2).reshape(N_CORES, P, (A // P) * D)

    tinit = np.zeros((T_rows, D), np.float32)
    tinit[slot_of[known_nodes]] = a[known_nodes, None] * np.asarray(
        x[known_nodes], np.float32
    )

    return dict(
        N=N, a=a, dest=dest, slab_nodes=dest, known_nodes=known_nodes,
        A=A, T_rows=T_rows, WR=WR, SWI=SWI,
        call_meta=call_meta, NCALL=NCALL, round_par=round_par,
        rounds_seq=rounds_seq, T1=T1, HT=HT,
        gidx=gidx, sidx=sidx, bexp=bexp, tinit=tinit,
    )


# ------------------------------------------------------------- bass builder


def _build_nc(plan, n_iters=N_ITERS):
    import concourse.bacc as bacc
    import concourse.mybir as mybir

    A = plan["A"]
    T_rows = plan["T_rows"]
    WR = plan["WR"]
    SWI = plan["SWI"]
    call_meta = plan["call_meta"]
    NCALL = plan["NCALL"]
    rounds_seq = plan["rounds_seq"]
    T1 = plan["T1"]
    HT = plan["HT"]
    f32, i16 = mybir.dt.float32, mybir.dt.int16
    GD = (A // P) * D  # free elems of accumulator tiles

    nc = bacc.Bacc(
        "TRN2", num_devices=N_CORES, detect_race_conditions=False,
        num_swdge_queues=4,
    )

    tinit = nc.declare_dram_parameter("tinit", [T_rows, D], f32, isOutput=False)
    gidx = nc.declare_dram_parameter("gidx", [P, SWI // 16], i16, isOutput=False)
    sidx = nc.declare_dram_parameter("sidx", [P, SWI // 16], i16, isOutput=False)
    bexp = nc.declare_dram_parameter("bexp", [P, GD], f32, isOutput=False)
    oslab = nc.declare_dram_parameter("oslab", [P, GD], f32, isOutput=True)

    table = nc.dram_tensor("table", [T_rows, D], f32, addr_space="Shared")
    HALF = A + P
    accum = nc.dram_tensor("accum", [2 * HALF, D], f32)
    bounce = nc.dram_tensor("bounce", [A, D], f32)

    RING = 16  # gathered-slot ring (call regions)
    NPAR0 = (NCALL + 1) // 2
    NPAR1 = NCALL // 2

    with (
        nc.sbuf_tensor("gtile", [P, RING * (CALLMAX // P) * D], f32) as gtile,
        nc.sbuf_tensor("rtile", [P, GD], f32) as rtile,
        nc.sbuf_tensor("htile", [P, GD], f32) as htile,
        nc.sbuf_tensor("btile", [P, GD], f32) as btile,
        nc.sbuf_tensor("ztile", [P, GD + D], f32) as ztile,
        nc.sbuf_tensor("gix", [P, SWI // 16], i16) as gix,
        nc.sbuf_tensor("six", [P, SWI // 16], i16) as six,
        nc.semaphore("isem") as isem,
        nc.semaphore("hsem") as hsem,
        nc.semaphore("gsemA") as gsemA,
        nc.semaphore("gsemB") as gsemB,
        nc.semaphore("ssemA") as ssemA,
        nc.semaphore("ssemB") as ssemB,
        nc.semaphore("zsem") as zsem,
        nc.semaphore("rbsem") as rbsem,
        nc.semaphore("vsem") as vsem,
        nc.semaphore("osem") as osem,
        nc.semaphore("csem") as csem,
        nc.Block() as block,
    ):
        # rounds: all scatter calls of round t are dest-disjoint (the t-th
        # edge of each dest) -> may run concurrently; across rounds the
        # same dest reappears -> serialize via ssem thresholds.
        first_call_of_round = {}
        round_idx = {rt: i for i, rt in enumerate(rounds_seq)}
        first_active_call = None
        for k, (r, t, w, n, o) in enumerate(call_meta):
            first_call_of_round.setdefault((r, t), k)
            if r == 1 and first_active_call is None:
                first_active_call = k
        if first_active_call is None:
            first_active_call = 0
        # per-parity cumulative call counts: npar[p][k] = #calls j<=k with j%2==p
        npar = [[0] * (NCALL + 1) for _ in range(2)]
        for k in range(NCALL):
            for p_ in range(2):
                npar[p_][k + 1] = npar[p_][k] + (1 if k % 2 == p_ else 0)
        NPAR = [npar[0][NCALL], npar[1][NCALL]]

        def slot_view(gk, n):
            base = (gk % RING) * (CALLMAX // P) * D
            W = n // P
            return gtile[:, base : base + W * D].rearrange("p (w d) -> p w d", d=D)

        @block.gpsimd
        def _(g):
            g.dma_start(gix[:], gidx[:]).then_inc(isem, 16)
            g.dma_start(six[:], sidx[:]).then_inc(isem, 16)
            g.dma_start(btile[:], bexp[:]).then_inc(isem, 16)
            g.memset(ztile[:], 0.0)
            g.wait_ge(isem, 48)
            g.wait_ge(hsem, 16 * 16)  # table initialized (16 chunks)

            def s_count(it, kend):
                # (threshA, threshB): scatters done among calls [0, kend) + it full iters
                return (
                    16 * (it * NPAR[0] + npar[0][kend]),
                    16 * (it * NPAR[1] + npar[1][kend]),
                )

            def emit_gather(it, k):
                r, t, w, n, off = call_meta[k]
                gk = it * NCALL + k
                if it > 0 and k == first_active_call:
                    g.wait_ge(csem, 2 * it)  # both AllGather pieces
                if gk >= RING:
                    # slot reuse WAR: scatter of call gk-RING (same parity) done
                    kprev = gk - RING
                    itp, kp = divmod(kprev, NCALL)
                    p_ = kp % 2
                    thr = 16 * (itp * NPAR[p_] + npar[p_][kp + 1])
                    g.wait_ge(ssemA if p_ == 0 else ssemB, thr)
                win = table[w * WR : min((w + 1) * WR, T_rows), :]
                p_ = k % 2
                g.dma_gather(
                    slot_view(gk, n), win,
                    gix[:, off // 16 : (off + n) // 16],
                    n, n, D, single_packet=False, queue_num=p_ * 2,
                ).then_inc(gsemA if p_ == 0 else gsemB, 16)

            def emit_scatter(it, k):
                r, t, w, n, off = call_meta[k]
                gk = it * NCALL + k
                p_ = k % 2
                g.wait_ge(
                    gsemA if p_ == 0 else gsemB,
                    16 * (it * NPAR[p_] + npar[p_][k + 1]),
                )
                ri = round_idx[(r, t)]
                if ri <= 1:
                    g.wait_ge(zsem, 32 * (it + 1))
                else:
                    # parity double-buffer: adjacent rounds use different
                    # accumulator halves; only rounds two back share ours
                    ta, tb = s_count(it, first_call_of_round[rounds_seq[ri - 1]])
                    g.wait_ge(ssemA, ta)
                    g.wait_ge(ssemB, tb)
                g.dma_scatter_add(
                    accum[:], slot_view(gk, n),
                    six[:, off // 16 : (off + n) // 16],
                    n, n, D, single_packet=False, queue_num=p_ * 2 + 1,
                ).then_inc(ssemA if p_ == 0 else ssemB, 16)

            for it in range(n_iters):
                if it > 0:
                    g.wait_ge(rbsem, 64 * it)  # accum consumed by readback
                # zero both accumulator halves (trash rows included)
                for h in range(2):
                    g.dma_start(
                        accum[h * HALF : (h + 1) * HALF, :].rearrange(
                            "(p q) d -> p (q d)", p=P
                        ),
                        ztile[:],
                    ).then_inc(zsem, 16)

                # software pipeline: gather k+1 issued before scatter k's
                # gsem wait.  Do NOT deepen this lag: each scatter's wait
                # throttles the SWDGE queues to ~2 outstanding 121-entry
                # calls; deeper lag overflows the 128-entry descriptor ring
                # on real HW (silent corruption -- sim only blocks).
                emit_gather(it, 0)
                for k in range(1, NCALL):
                    emit_gather(it, k)
                    emit_scatter(it, k - 1)
                emit_scatter(it, NCALL - 1)

                if it < n_iters - 1:
                    for pc in range(2):
                        g.wait_ge(osem, 32 * it + 16 * (pc + 1))
                        g.collective_compute(
                            "AllGather",
                            mybir.AluOpType.bypass,
                            replica_groups=[list(range(N_CORES))],
                            ins=[bounce[pc * HT : (pc + 1) * HT, :]],
                            outs=[
                                table[
                                    pc * N_CORES * HT : (pc + 1) * N_CORES * HT, :
                                ]
                            ],
                        ).then_inc(csem, 1)
            g.wait_ge(osem, 32 * n_iters)

        @block.vector
        def _(v):
            v.wait_ge(isem, 48)  # btile loaded
            for it in range(n_iters):
                for pc, (p0, p1) in enumerate(((0, 64), (64, P))):
                    v.wait_ge(rbsem, 64 * it + 32 * (pc + 1))
                    v.tensor_add(
                        rtile[p0:p1, :], rtile[p0:p1, :], htile[p0:p1, :]
                    )
                    v.tensor_mul(
                        rtile[p0:p1, :], rtile[p0:p1, :], btile[p0:p1, :]
                    ).then_inc(vsem, 1)

        @block.sync
        def _(s):
            NCH = 16
            rows = -(-T_rows // NCH)
            while rows * 8 >= 65536:
                NCH *= 2
                rows = -(-T_rows // NCH)
            for ch in range(NCH):
                r0 = ch * rows
                r1 = min((ch + 1) * rows, T_rows)
                if r0 < r1:
                    s.dma_start(table[r0:r1, :], tinit[r0:r1, :]).then_inc(hsem, 16)
            kend1 = first_call_of_round.get((1, T1), NCALL)
            ta1_0 = npar[0][kend1]
            ta1_1 = npar[1][kend1]
            for it in range(n_iters):
                for pc in range(2):
                    if pc == 0:
                        s.wait_ge(ssemA, 16 * (it * NPAR0 + ta1_0))
                        s.wait_ge(ssemB, 16 * (it * NPAR1 + ta1_1))
                    else:
                        s.wait_ge(ssemA, 16 * NPAR0 * (it + 1))
                        s.wait_ge(ssemB, 16 * NPAR1 * (it + 1))
                    if it > 0:
                        s.wait_ge(osem, 32 * it)  # rtile free
                    p0 = pc * 64
                    r0 = pc * HT
                    s.dma_start(
                        rtile[p0 : p0 + 64, :],
                        accum[r0 : r0 + HT, :].rearrange(
                            "(p q) d -> p (q d)", p=64
                        ),
                    ).then_inc(rbsem, 16)
                    s.dma_start(
                        htile[p0 : p0 + 64, :],
                        accum[HALF + r0 : HALF + r0 + HT, :].rearrange(
                            "(p q) d -> p (q d)", p=64
                        ),
                    ).then_inc(rbsem, 16)
                    s.wait_ge(vsem, 2 * it + pc + 1)
                    if it < n_iters - 1:
                        if it > 0 or pc > 0:
                            s.wait_ge(csem, 2 * it + pc - 1 if it > 0 else 0)
                        if it > 0:
                            s.wait_ge(csem, 2 * (it - 1) + pc + 1)
                        dst = bounce[r0 : r0 + HT, :].rearrange(
                            "(p q) d -> p (q d)", p=64
                        )
                    else:
                        dst = oslab[p0 : p0 + 64, :]
                    s.dma_start(dst, rtile[p0 : p0 + 64, :]).then_inc(osem, 16)

    return nc


# ------------------------------------------------------------------ runner


LAST_EXEC_TIME_NS = None
LAST_RESULT = None


def kernel(x, edge_index, known_feature_mask):
    global LAST_EXEC_TIME_NS, LAST_RESULT
    from concourse.bass_utils import run_bass_kernel_spmd

    x = np.asarray(x, np.float32)
    edge_index = np.asarray(edge_index)
    known_feature_mask = np.asarray(known_feature_mask)

    plan = _prepare(x, edge_index, known_feature_mask)
    nc = _build_nc(plan)
    nc.compile()

    in_maps = [
        {
            "tinit": plan["tinit"],
            "gidx": np.ascontiguousarray(plan["gidx"][c]),
            "sidx": np.ascontiguousarray(plan["sidx"][c]),
            "bexp": np.ascontiguousarray(plan["bexp"][c]),
        }
        for c in range(N_CORES)
    ]
    res = run_bass_kernel_spmd(nc, in_maps, core_ids=list(range(N_CORES)))
    LAST_RESULT = res

    N = plan["N"]
    a = plan["a"]
    dest = plan["dest"]
    A = plan["A"]
    out_full = np.zeros((N, D), np.float32)
    for c in range(N_CORES):
        oslab = np.asarray(res.results[c]["oslab"]).reshape(A, D)
        nodes = dest[c]
        m = nodes >= 0
        nn = nodes[m]
        out_full[nn] = oslab[m] / a[nn, None]
    kn = plan["known_nodes"]
    out_full[kn] = x[kn]
    return out_full

